# revision 42
# baseline (speedup 1.0000x reference)
"""Causal multi-head self-attention on 8 TRN2 NeuronCores.

Sharding: tensor-parallel over heads. 16 heads / 8 cores = 2 heads per core.
Each core computes q/k/v projections for its 2 heads (feature-major via
fp32r matmuls), block-causal attention (scores kept k-major so softmax sums
come from a fused ones-column in the attn@v matmul and no transposes are
needed), and a partial output projection against its 128-column slice of
W_O. The host sums the 8 partial outputs.

Layouts on core c (heads 2c, 2c+1 = "A", "B"):
  qT/kT  [128, 2048]  feature-major; rows 0:64 head A dk, 64:128 head B
  vtok   [128, 16, 192] token-major v (PE-transposed): cols 0:64 vA, col 64
         a shared ones column, cols 128:192 vB. Head A's ctx lhsT slice is
         cols 0:128 (ctxA on partitions 0:64, sumA on 64); head B's slice is
         cols 64:192, so the same ones column becomes its col 0 (sumB on
         partition 0) and vB lands on partitions 64:128 — no partition-shift
         DMA is needed to assemble ctxn.
  scoresT[128 k-tok, <=512 q-tok] per (q-tile, k-tile); exp'd on ScalarE
  ctxAB  [128, 2*512] PSUM; A half rows 0:64 + sum row 64, B half sum row 0
         + rows 64:128
  out    partial [1024, 8192] feature-major; host sums over cores + transposes

Pipelining: the whole kernel is one deferred-work machine. Attention of
batch b interleaves (as pop-closures in its k-tile loop) the q/k/v
projections + v-transposes of batch b+1, plus the softmax-normalization
(finish_rb) and output-projection (finish_oproj) of the previous q-tile.
This keeps the PE stream dense (pstate stays at max clock) and gives the
scalar engine's exp stream (~1us per k-iter, the attention pacer) slack.

Softmax denominators: sumA sits on partition 64, sumB on partition 0 of the
two ctx halves; both are same-partition scalar copies into s2, whose rows
1:63 hold persistent zeros, and a K=65 indicator matmul broadcasts them to
[128, QTILE]. reciprocal_approx_fast (~18 bits) + one DVE multiply
normalize ctx.

Toolchain constraints honored here: col-offset tile_position is illegal for
4-byte matmul dtypes; fp32r consumers need fp32r-typed producers; x is
transposed on the host so every DMA has a contiguous innermost run (>=2KB);
fp32r matmuls with moving dim >= 256 run at full PE rate.
"""

import numpy as np
from contextlib import ExitStack

import concourse.bass as bass
import concourse.tile as tile
from concourse import bacc, mybir
from concourse.bass_utils import run_bass_kernel_spmd

F32 = mybir.dt.float32
F32R = mybir.dt.float32r
BF16 = mybir.dt.bfloat16
FP16 = mybir.dt.float16

B, S, D, H = 4, 2048, 1024, 16
DK = D // H  # 64
NCORES = 8
T = B * S  # 8192 tokens
KT = D // 128  # 8 contraction tiles for projections
QTILE = 512  # q-tile width (tokens)
KTILE = 128  # k-tile width (tokens)
NQT = S // QTILE  # 4 q-tiles per batch
NKT = S // KTILE  # 16 k-tiles per batch
LAG = 3  # ctx matmuls trail scores by this many k-tiles
N_WARM = 6  # PE warm-up garbage matmuls at kernel start
EXP_FUNC = mybir.ActivationFunctionType.Exp
INV_SQRT_DK = 1.0 / np.sqrt(DK)


def build_nc():
    nc = bacc.Bacc("TRN2", target_bir_lowering=False, debug=False)

    # host pre-layouts so every DMA runs long-contiguous per partition:
    # x [128, 16 tiles, KT, 512] -> 8KB runs; weights [128, KT, 128] -> 2KB;
    # output [128, 16, KT, 512] -> 4KB-per-half runs (host inverts).
    NT = T // QTILE  # 16 global q-tiles
    xT = nc.dram_tensor("xT", [128, NT, KT, QTILE], BF16, kind="ExternalInput").ap()
    wq = nc.dram_tensor("wq", [128, KT, 128], BF16, kind="ExternalInput").ap()
    wk = nc.dram_tensor("wk", [128, KT, 128], BF16, kind="ExternalInput").ap()
    wv = nc.dram_tensor("wv", [128, KT, 128], BF16, kind="ExternalInput").ap()
    wo = nc.dram_tensor("wo", [128, D], F32, kind="ExternalInput").ap()
    tri = nc.dram_tensor("tri", [128, 128], BF16, kind="ExternalInput").ap()
    ind = nc.dram_tensor("ind", [65, 128], F32, kind="ExternalInput").ap()
    ident = nc.dram_tensor("ident", [128, 128], BF16, kind="ExternalInput").ap()
    outT = nc.dram_tensor("outT", [128, NT, KT, QTILE], FP16, kind="ExternalOutput").ap()

    with ExitStack() as ctx:
        tc = ctx.enter_context(tile.TileContext(nc))
        consts = ctx.enter_context(tc.tile_pool(name="consts", bufs=1))
        xt_pool = ctx.enter_context(tc.tile_pool(name="xt_pool", bufs=3))
        batch_pool = ctx.enter_context(tc.tile_pool(name="batch_pool", bufs=2))
        vtmp_pool = ctx.enter_context(tc.tile_pool(name="vtmp_pool", bufs=3))
        exp_pool = ctx.enter_context(tc.tile_pool(name="exp_pool", bufs=5))
        ctxn_pool = ctx.enter_context(tc.tile_pool(name="ctxn_pool", bufs=3))
        oall_pool = ctx.enter_context(tc.tile_pool(name="oall_pool", bufs=2))
        small_pool = ctx.enter_context(tc.tile_pool(name="small_pool", bufs=3))
        ps = ctx.enter_context(tc.tile_pool(name="ps", bufs=1, space="PSUM"))

        # --- PE warm-up ---
        # The HW activity monitor starts the PE in a half-rate state and only
        # promotes after sustained matmul activity; garbage matmuls during the
        # initial DMA wait start that clock early so the real projections run
        # at full rate sooner.
        warmS = consts.tile([128, QTILE], F32R, name="warmS")
        nc.gpsimd.memset(warmS.bitcast(F32), 1.0)

        def dummy_mm(n=1):
            # tag "mm": those buffers recycle fast (their consumers are plain
            # copies), so a filler never parks the in-order PE queue behind a
            # slow consumer the way an sAB buffer (exp-gated) would.
            for _ in range(n):
                dP = ps.tile([128, QTILE], F32, name="oP", tag="mm", bufs=2)
                nc.tensor.matmul(
                    dP, warmS[:, 0:128], warmS, start=True, stop=True
                )

        def dummy_sc(n=1):
            # sAB-bank filler for use INSIDE the oproj drain loop, where an
            # "mm" filler would rotate oP's 2 buffers and break its
            # matmul/copy double-buffering. Only safe once attention is over
            # (the sAB bank's exp has long completed).
            for _ in range(n):
                dP = ps.tile([128, 4 * QTILE], F32, name="sAB", tag="sc", bufs=1)
                nc.tensor.matmul(
                    dP[:, 0:QTILE], warmS[:, 0:128], warmS, start=True, stop=True
                )

        dummy_mm(N_WARM)
        # closures consult this at call time: the final drain sets it so
        # oproj emits PE filler between its copy-paced matmuls
        dense_drain = [False]

        # --- constants / weights (persistent) ---
        # Ordered so the head of the kernel only waits for wq + the first
        # x chunk: wo (first needed ~40us in) and small consts go later.
        # wq in one 256KB DMA, then the first x tile in 2-kt chunks: each
        # descriptor costs ~600ns to issue, so fewer+bigger beats per-kt
        # interleave; a 256KB chunk feeds two matmuls and the stream stays
        # dense from the first dummy onward (any PE gap resets the activity
        # monitor's ~4.5us promotion clock).
        wq_sb = consts.tile([128, KT, 128], BF16)
        nc.sync.dma_start(out=wq_sb, in_=wq)
        xt00 = xt_pool.tile([128, KT, QTILE], BF16, name="xt", tag="xt")
        for k0, k1 in ((0, 3), (3, 6), (6, 8)):
            nc.sync.dma_start(out=xt00[:, k0:k1, :], in_=xT[:, 0, k0:k1, :])
        wk_sb = consts.tile([128, KT, 128], BF16)
        nc.sync.dma_start(out=wk_sb, in_=wk)
        wv_sb = consts.tile([128, KT, 128], BF16)
        nc.sync.dma_start(out=wv_sb, in_=wv)
        tri_sb = consts.tile([128, 128], BF16)
        nc.sync.dma_start(out=tri_sb, in_=tri)
        ind_sb = consts.tile([65, 128], F32R)
        nc.sync.dma_start(out=ind_sb, in_=ind.bitcast(F32R))
        ident_sb = consts.tile([128, 128], BF16)
        nc.sync.dma_start(out=ident_sb, in_=ident)
        wo_sb = consts.tile([128, KT, 128], F32R)
        nc.sync.dma_start(
            out=wo_sb, in_=wo.rearrange("p (jt m) -> p jt m", jt=KT).bitcast(F32R)
        )

        finishq = []  # [finish_rb(qi), finish_oproj(qi)] of the prev q-tile
        projq = []  # stage-A closures of the NEXT batch

        def build_stage_a(b):
            """Allocate batch b's persistent tiles and return them with the
            list of closures that emit its projection work."""
            tb = b * S
            qT_sb = batch_pool.tile([128, S], BF16, name="qT_sb")
            # kT2 half 0: [kA; 0], half 1: [0; kB] — full-K scores keep the
            # whole PE array active so HAM stays at full clock. Zero halves
            # are initialized on the first visit of each pool buffer (b<2)
            # and inherited afterwards.
            kT2_sb = batch_pool.tile([128, 2, S], BF16, name="kT2_sb")
            vtok_sb = batch_pool.tile([128, NKT, 192], BF16, name="vtok_sb")
            xts = [
                xt00
                if (b == 0 and tt == 0)
                else xt_pool.tile([128, KT, QTILE], BF16, name="xt", tag="xt")
                for tt in range(NQT)
            ]

            issued = set()

            def prefetch(tt, b=b, xts=xts, issued=issued):
                if tt in issued or (b == 0 and tt == 0):
                    return  # only issue each tile's DMA once
                issued.add(tt)
                g = b * NQT + tt
                nc.sync.dma_start(out=xts[tt], in_=xT[:, g, :, :])

            closures = []
            if b < 2:

                def init_consts(kT2_sb=kT2_sb, vtok_sb=vtok_sb):
                    # ones column + kT2 zero halves; GpSimd is otherwise idle
                    nc.gpsimd.memset(vtok_sb[:, :, 64:65], 1.0)
                    nc.gpsimd.memset(kT2_sb[64:128, 0, :], 0.0)
                    nc.gpsimd.memset(kT2_sb[0:64, 1, :], 0.0)

                closures.append(init_consts)

            for tt in range(NQT):
                cell = {}

                def proj_q(tt=tt, qT_sb=qT_sb, xts=xts, cell=cell):
                    xt = xts[tt]
                    cell["xt"] = xt
                    qP = ps.tile([128, QTILE], F32, name="qP", tag="mm", bufs=2)
                    for kt in range(KT):
                        nc.tensor.matmul(
                            qP,
                            wq_sb[:, kt, :],
                            xt[:, kt, :],
                            start=(kt == 0),
                            stop=(kt == KT - 1),
                        )
                    nc.vector.tensor_copy(
                        qT_sb[:, tt * QTILE : (tt + 1) * QTILE], qP
                    )

                def proj_k(tt=tt, kT2_sb=kT2_sb, cell=cell):
                    xt = cell["xt"]
                    kP = ps.tile([128, QTILE], F32, name="kP", tag="mm", bufs=2)
                    for kt in range(KT):
                        nc.tensor.matmul(
                            kP,
                            wk_sb[:, kt, :],
                            xt[:, kt, :],
                            start=(kt == 0),
                            stop=(kt == KT - 1),
                        )
                    nc.vector.tensor_copy(
                        kT2_sb[0:64, 0, tt * QTILE : (tt + 1) * QTILE], kP[0:64, :]
                    )
                    nc.vector.tensor_copy(
                        kT2_sb[64:128, 1, tt * QTILE : (tt + 1) * QTILE],
                        kP[64:128, :],
                    )

                def proj_v(tt=tt, prefetch=prefetch, cell=cell):
                    xt = cell["xt"]
                    vP = ps.tile([128, QTILE], F32, name="vP", tag="mm", bufs=2)
                    for kt in range(KT):
                        nc.tensor.matmul(
                            vP,
                            wv_sb[:, kt, :],
                            xt[:, kt, :],
                            start=(kt == 0),
                            stop=(kt == KT - 1),
                        )
                    vT_tmp = vtmp_pool.tile([128, QTILE], BF16, name="vT_tmp")
                    nc.vector.tensor_copy(vT_tmp, vP)
                    cell["vT"] = vT_tmp
                    if tt + 3 < NQT:
                        # xt buffer for tt is free after this group; refill
                        prefetch(tt + 3)

                def vtrans(tt=tt, vtok_sb=vtok_sb, cell=cell):
                    vT_tmp = cell["vT"]
                    for s in range(QTILE // 128):
                        vtokP = ps.tile(
                            [128, 128], BF16, name="vtokP", tag="mm", bufs=2
                        )
                        nc.tensor.transpose(
                            vtokP, vT_tmp[:, s * 128 : (s + 1) * 128], ident_sb
                        )
                        m = tt * 4 + s
                        nc.vector.tensor_copy(vtok_sb[:, m, 0:64], vtokP[:, 0:64])
                        nc.vector.tensor_copy(
                            vtok_sb[:, m, 128:192], vtokP[:, 64:128]
                        )

                closures += [proj_q, proj_k, proj_v, vtrans]
            return (qT_sb, kT2_sb, vtok_sb), prefetch, closures

        def attention(b, tiles):
            qT_sb, kT2_sb, vtok_sb = tiles
            tb = b * S
            for qi in range(NQT):
                q0 = qi * QTILE  # batch-local q base
                nk = 4 * qi + 4  # k-tiles for this q-tile (block-causal)
                ctxAB = ps.tile(
                    [128, 2 * QTILE], F32, name="ctxAB", tag="ctx", bufs=1
                )

                def geom(m, qi=qi):
                    d_off = m - 4 * qi
                    if d_off >= 0:
                        return QTILE - 128 * d_off, 128 * d_off, True
                    return QTILE, 0, False

                # finish pops: rb early (i=2), oproj spread so the last half
                # lands in the LAG-flush slots where the PE would otherwise
                # idle behind exp-gated ctx matmuls; projq keeps 4 groups in
                # reserve for the same flush region (by tile end the exp
                # stream lags the scores, so every flush ctx waits ~1us).
                finish_slots = sorted({2, 5, nk - 1, nk + 1})
                # scores for a PAIR of k-tiles share one 4-bank PSUM tile
                # laid out [A0 | B0 | A1 | B1]; ONE exp covers the pair, so
                # the scalar engine pays its ~380ns per-instruction overhead
                # half as often (the exp stream is the attention pacer).
                exps = {}
                cur = {}
                for i in range(nk + LAG):
                    popped = False
                    if i in finish_slots and finishq:
                        finishq.pop(0)()
                        popped = True
                    elif (
                        projq
                        and (i % 2 == 1 or i >= 10 or i >= nk)
                        and (len(projq) > 4 or i >= nk - 3)
                    ):
                        projq.pop(0)()
                        popped = True
                    if not popped and i >= nk - 3:
                        dummy_mm(1)
                    if i < nk:
                        m = i
                        width, qoff, diag = geom(m)
                        if m % 2 == 0:
                            cur["s"] = ps.tile(
                                [128, 4 * QTILE], F32, name="sAB", tag="sc", bufs=1
                            )
                        sAB = cur["s"]
                        base = 2 * (m % 2) * QTILE
                        nc.tensor.matmul(
                            sAB[:, base : base + width],
                            kT2_sb[:, 0, m * 128 : (m + 1) * 128],
                            qT_sb[:, q0 + qoff : q0 + QTILE],
                            start=True,
                            stop=True,
                        )
                        nc.tensor.matmul(
                            sAB[:, base + QTILE : base + QTILE + width],
                            kT2_sb[:, 1, m * 128 : (m + 1) * 128],
                            qT_sb[:, q0 + qoff : q0 + QTILE],
                            start=True,
                            stop=True,
                        )
                        if m % 2 == 1:
                            # widths shrink with m, so quarters 0-2 are full
                            # and only the last quarter's tail is junk-free
                            eAB = exp_pool.tile(
                                [128, 4 * QTILE], BF16, name="eAB", tag="exp"
                            )
                            nc.scalar.activation(
                                eAB[:, 0 : 3 * QTILE + width],
                                sAB[:, 0 : 3 * QTILE + width],
                                EXP_FUNC,
                                scale=INV_SQRT_DK,
                            )
                            for half_idx, m_sub in ((0, m - 1), (1, m)):
                                if not geom(m_sub)[2]:
                                    continue  # not a diagonal k-tile
                                for head in range(2):
                                    c0 = (2 * half_idx + head) * QTILE
                                    nc.vector.tensor_mul(
                                        eAB[:, c0 : c0 + 128],
                                        eAB[:, c0 : c0 + 128],
                                        tri_sb,
                                    )
                            exps[m // 2] = eAB

                    j = i - LAG
                    if j >= 0:
                        width, qoff, _ = geom(j)
                        first = j == 0
                        last = j == nk - 1
                        eAB = exps[j // 2] if j % 2 == 0 else exps.pop(j // 2)
                        base = 2 * (j % 2) * QTILE
                        nc.tensor.matmul(
                            ctxAB[:, qoff:QTILE],
                            vtok_sb[:, j, 0:128],
                            eAB[:, base : base + width],
                            start=first,
                            stop=last,
                            skip_group_check=True,
                        )
                        nc.tensor.matmul(
                            ctxAB[:, QTILE + qoff : 2 * QTILE],
                            vtok_sb[:, j, 64:192],
                            eAB[:, base + QTILE : base + QTILE + width],
                            start=first,
                            stop=last,
                            skip_group_check=True,
                        )

                # normalization part 1 (immediate, frees the ctx PSUM slots):
                # sumB is already on partition 0 of the B half (shared ones
                # col) and sumA on partition 64 of the A half — both are
                # same-partition scalar copies into s2; rows 1:63 of s2 hold
                # zeros (memset once per pool buffer) so the K=65 broadcast
                # matmul ignores the junk. ctx copies go on the vector
                # engine so the scalar EXP stream is not delayed.
                s2 = small_pool.tile([65, 2, QTILE], F32R, name="s2")
                if b == 0 and qi < 3:
                    nc.gpsimd.memset(s2[:, 0, :].bitcast(F32), 0.0)
                nc.vector.tensor_copy(s2[0:1, 0, :], ctxAB[0:1, QTILE : 2 * QTILE])
                nc.vector.tensor_copy(s2[64:65, 0, :], ctxAB[64:65, 0:QTILE])
                ctxn = ctxn_pool.tile([128, QTILE], F32R, name="ctxn")
                nc.vector.tensor_copy(ctxn[0:64, :], ctxAB[0:64, 0:QTILE])
                nc.vector.tensor_copy(
                    ctxn[64:128, :], ctxAB[64:128, QTILE : 2 * QTILE]
                )

                def finish_rb(qi=qi, ctxn=ctxn, s2=s2):
                    # deferred stage 1: K=65 indicator matmul broadcasts both
                    # sums to [128, QTILE] PSUM (row 0 = sumB -> partitions
                    # 64:128, row 64 = sumA -> 0:64); fast-approx reciprocal
                    # + normalize.
                    rbP = ps.tile([128, QTILE], F32, name="rbP", tag="mm", bufs=2)
                    nc.tensor.matmul(
                        rbP, ind_sb, s2[0:65, 0, :], start=True, stop=True
                    )
                    rb_sb = small_pool.tile([128, QTILE], F32, name="rb_sb")
                    nc.vector.reciprocal_approx_fast(out=rb_sb, in_=rbP)
                    nc.vector.tensor_mul(ctxn, ctxn, rb_sb.bitcast(F32R))

                cell = {}

                def oproj_half(h, qi=qi, q0=q0, tb=tb, ctxn=ctxn, cell=cell):
                    # deferred stage 2: output projection (ctxn is normalized
                    # by the time this runs several m-iterations later), in
                    # two pop-halves so attention matmuls interleave with the
                    # copy-paced PSUM drain.
                    if h == 0:
                        cell["o_all"] = oall_pool.tile(
                            [128, KT, QTILE], FP16, name="o_all"
                        )
                    o_all = cell["o_all"]
                    jt0, jt1 = h * (KT // 2), (h + 1) * (KT // 2)
                    g = (tb + q0) // QTILE
                    odst = outT[:, g, jt0:jt1, :]
                    engs = [nc.scalar.copy, nc.vector.tensor_copy]
                    for jt in range(jt0, jt1):
                        oP = ps.tile([128, QTILE], F32, name="oP", tag="mm", bufs=2)
                        nc.tensor.matmul(
                            oP, wo_sb[:, jt, :], ctxn, start=True, stop=True
                        )
                        engs[jt % 2](o_all[:, jt, :], oP)
                        if dense_drain[0]:
                            # in the final drain these matmuls pace at the
                            # copy cadence (~50% PE); filler between them
                            # keeps the activity monitor at full rate
                            dummy_sc(1)
                    # per-half DMA: halves the final drain's trailing transfer
                    nc.sync.dma_start(out=odst, in_=o_all[:, jt0:jt1, :])

                finishq.extend(
                    [
                        finish_rb,
                        lambda f=oproj_half: f(0),
                        lambda f=oproj_half: f(1),
                    ]
                )

        # batch 0's stage A runs inline (nothing to overlap with); its xt
        # prefetches ride the pool rotation as the closures execute.
        tiles0, prefetch0, cl0 = build_stage_a(0)
        for tt in range(1, NQT):
            prefetch0(tt)
        for c in cl0:
            c()
        tiles = tiles0
        for b in range(B):
            if b + 1 < B:
                next_tiles, next_prefetch, next_cl = build_stage_a(b + 1)
                # keep the xt pipeline 3 tiles deep across the batch seam
                for tt in range(min(3, NQT)):
                    next_prefetch(tt)
                projq.extend(next_cl)
            attention(b, tiles)
            while projq:
                projq.pop(0)()
            if b + 1 < B:
                tiles = next_tiles

        # final drain: the copy/recip chains leave the PE ~40% idle, which
        # trips the activity monitor into half-rate mode and stretches the
        # remaining matmuls 2x. Garbage matmuls keep it hot; they go BEFORE
        # each pop because the PE queue is in-order — filler emitted after a
        # stalled real matmul can never run during that stall.
        dense_drain[0] = True
        while finishq:
            dummy_mm(3)
            finishq.pop(0)()
        dummy_mm(2)

    nc.compile()
    return nc


_NC = None


def _get_nc():
    global _NC
    if _NC is None:
        _NC = build_nc()
    return _NC


def make_in_maps(x, W_Q, W_K, W_V, W_O):
    import ml_dtypes

    NT = T // QTILE
    # x and the q/k/v weights ship as bf16: halves the 32MB/core input DMA
    # (q/k/v are consumed as bf16 downstream anyway; verified rel err 2.6e-3
    # vs the 2e-2 gate). W_O stays fp32 for the fp32r output-projection path.
    # Layouts are pre-tiled on the host so each DMA reads 2-8KB contiguous
    # runs per partition: x -> [p, tile, kt, t], w -> [p, kt, m].
    xT = np.asarray(x, dtype=np.float32).reshape(T, D).T.astype(ml_dtypes.bfloat16)
    xTh = np.ascontiguousarray(
        xT.reshape(KT, 128, NT, QTILE).transpose(1, 2, 0, 3)
    )
    W_Q = np.asarray(W_Q, dtype=np.float32).astype(ml_dtypes.bfloat16)
    W_K = np.asarray(W_K, dtype=np.float32).astype(ml_dtypes.bfloat16)
    W_V = np.asarray(W_V, dtype=np.float32).astype(ml_dtypes.bfloat16)
    W_O = np.asarray(W_O, dtype=np.float32)

    def wtile(W, sl):
        # [p, kt, m] with m contiguous per (p, kt): W[sl].T is [D, 128]
        return np.ascontiguousarray(
            W[sl, :].T.reshape(KT, 128, 128).transpose(1, 0, 2)
        )

    tri = np.triu(np.ones((128, 128), dtype=ml_dtypes.bfloat16))  # tri[k,q]=1 iff q>=k
    ind2 = np.zeros((65, 128), dtype=np.float32)
    ind2[0, 64:128] = 1.0  # row 0 = sumB -> partitions 64:128
    ind2[64, 0:64] = 1.0  # row 64 = sumA -> partitions 0:64
    ident = np.eye(128, dtype=ml_dtypes.bfloat16)
    in_maps = []
    for c in range(NCORES):
        sl = slice(c * 128, (c + 1) * 128)
        in_maps.append(
            {
                "xT": xTh,
                "wq": wtile(W_Q, sl),
                "wk": wtile(W_K, sl),
                "wv": wtile(W_V, sl),
                "wo": np.ascontiguousarray(W_O.T[sl, :]),
                "tri": tri,
                "ind": ind2,
                "ident": ident,
            }
        )
    return in_maps


def kernel(x, W_Q, W_K, W_V, W_O, _results_hook=None):
    nc = _get_nc()
    in_maps = make_in_maps(x, W_Q, W_K, W_V, W_O)
    res = run_bass_kernel_spmd(nc, in_maps, list(range(NCORES)))
    if _results_hook is not None:
        _results_hook(res)
    NT = T // QTILE
    acc = np.zeros((128, NT, KT, QTILE), dtype=np.float64)
    for c in range(NCORES):
        acc += res.results[c]["outT"]
    # [p, g, jt, t] -> outT[jt*128+p, g*512+t] -> [T, D] -> [B, S, D]
    outT = acc.transpose(2, 0, 1, 3).reshape(D, T)
    out = np.ascontiguousarray(outT.T).reshape(B, S, D).astype(np.float32)
    return out



# revision 45
# speedup vs baseline: 1.1763x; 1.1763x over previous
"""Causal multi-head self-attention on 8 TRN2 NeuronCores.

Sharding: tensor-parallel over heads. 16 heads / 8 cores = 2 heads per core.
Each core computes q/k/v projections for its 2 heads (feature-major via
fp32r matmuls), block-causal attention (scores kept k-major so softmax sums
come from a fused ones-column in the attn@v matmul and no transposes are
needed), and a partial output projection against its 128-column slice of
W_O. The host sums the 8 partial outputs.

Layouts on core c (heads 2c, 2c+1 = "A", "B"):
  qT/kT  [128, 2048]  feature-major; rows 0:64 head A dk, 64:128 head B
  vtok   [128, 16, 192] token-major v (PE-transposed): cols 0:64 vA, col 64
         a shared ones column, cols 128:192 vB. Head A's ctx lhsT slice is
         cols 0:128 (ctxA on partitions 0:64, sumA on 64); head B's slice is
         cols 64:192, so the same ones column becomes its col 0 (sumB on
         partition 0) and vB lands on partitions 64:128 — no partition-shift
         DMA is needed to assemble ctxn.
  scoresT[128 k-tok, <=512 q-tok] per (q-tile, k-tile); exp'd on ScalarE
  ctxAB  [128, 2*512] PSUM; A half rows 0:64 + sum row 64, B half sum row 0
         + rows 64:128
  out    partial [1024, 8192] feature-major; host sums over cores + transposes

Pipelining: the whole kernel is one deferred-work machine. Attention of
batch b interleaves (as pop-closures in its k-tile loop) the q/k/v
projections + v-transposes of batch b+1, plus the softmax-normalization
(finish_rb) and output-projection (finish_oproj) of the previous q-tile.
This keeps the PE stream dense (pstate stays at max clock) and gives the
scalar engine's exp stream (~1us per k-iter, the attention pacer) slack.

Softmax denominators: sumA sits on partition 64, sumB on partition 0 of the
two ctx halves; both are same-partition scalar copies into s2, whose rows
1:63 hold persistent zeros, and a K=65 indicator matmul broadcasts them to
[128, QTILE]. reciprocal_approx_fast (~18 bits) + one DVE multiply
normalize ctx.

Toolchain constraints honored here: col-offset tile_position is illegal for
4-byte matmul dtypes; fp32r consumers need fp32r-typed producers; x is
transposed on the host so every DMA has a contiguous innermost run (>=2KB);
fp32r matmuls with moving dim >= 256 run at full PE rate.
"""

import numpy as np
from contextlib import ExitStack

import concourse.bass as bass
import concourse.tile as tile
from concourse import bacc, mybir
from concourse.bass_utils import run_bass_kernel_spmd

F32 = mybir.dt.float32
F32R = mybir.dt.float32r
BF16 = mybir.dt.bfloat16
FP16 = mybir.dt.float16

B, S, D, H = 4, 2048, 1024, 16
DK = D // H  # 64
NCORES = 8
T = B * S  # 8192 tokens
KT = D // 128  # 8 contraction tiles for projections
QTILE = 512  # q-tile width (tokens)
KTILE = 128  # k-tile width (tokens)
NQT = S // QTILE  # 4 q-tiles per batch
NKT = S // KTILE  # 16 k-tiles per batch
LAG = 2  # ctx matmuls trail scores by this many k-tiles
N_WARM = 6  # PE warm-up garbage matmuls at kernel start
EXP_FUNC = mybir.ActivationFunctionType.Exp
INV_SQRT_DK = 1.0 / np.sqrt(DK)


def build_nc():
    nc = bacc.Bacc("TRN2", target_bir_lowering=False, debug=False)

    # host pre-layouts so every DMA runs long-contiguous per partition:
    # x [128, 16 tiles, KT, 512] -> 8KB runs; weights [128, KT, 128] -> 2KB;
    # output [128, 16, KT, 512] -> 4KB-per-half runs (host inverts).
    NT = T // QTILE  # 16 global q-tiles
    xT = nc.dram_tensor("xT", [128, NT, KT, QTILE], BF16, kind="ExternalInput").ap()
    wq = nc.dram_tensor("wq", [128, KT, 128], BF16, kind="ExternalInput").ap()
    wk = nc.dram_tensor("wk", [128, KT, 128], BF16, kind="ExternalInput").ap()
    wv = nc.dram_tensor("wv", [128, KT, 128], BF16, kind="ExternalInput").ap()
    wo = nc.dram_tensor("wo", [128, D], F32, kind="ExternalInput").ap()
    tri = nc.dram_tensor("tri", [128, 128], BF16, kind="ExternalInput").ap()
    ind = nc.dram_tensor("ind", [65, 128], F32, kind="ExternalInput").ap()
    ident = nc.dram_tensor("ident", [128, 128], BF16, kind="ExternalInput").ap()
    outT = nc.dram_tensor("outT", [128, NT, KT, QTILE], FP16, kind="ExternalOutput").ap()

    with ExitStack() as ctx:
        tc = ctx.enter_context(tile.TileContext(nc))
        consts = ctx.enter_context(tc.tile_pool(name="consts", bufs=1))
        xt_pool = ctx.enter_context(tc.tile_pool(name="xt_pool", bufs=3))
        batch_pool = ctx.enter_context(tc.tile_pool(name="batch_pool", bufs=2))
        vtmp_pool = ctx.enter_context(tc.tile_pool(name="vtmp_pool", bufs=3))
        exp_pool = ctx.enter_context(tc.tile_pool(name="exp_pool", bufs=5))
        ctxn_pool = ctx.enter_context(tc.tile_pool(name="ctxn_pool", bufs=3))
        oall_pool = ctx.enter_context(tc.tile_pool(name="oall_pool", bufs=2))
        small_pool = ctx.enter_context(tc.tile_pool(name="small_pool", bufs=3))
        ps = ctx.enter_context(tc.tile_pool(name="ps", bufs=1, space="PSUM"))

        # --- PE warm-up ---
        # The HW activity monitor starts the PE in a half-rate state and only
        # promotes after sustained matmul activity; garbage matmuls during the
        # initial DMA wait start that clock early so the real projections run
        # at full rate sooner.
        warmS = consts.tile([128, QTILE], F32R, name="warmS")
        nc.gpsimd.memset(warmS.bitcast(F32), 1.0)

        def dummy_mm(n=1):
            # tag "mm": those buffers recycle fast (their consumers are plain
            # copies), so a filler never parks the in-order PE queue behind a
            # slow consumer the way an sAB buffer (exp-gated) would.
            for _ in range(n):
                dP = ps.tile([128, QTILE], F32, name="oP", tag="mm", bufs=2)
                nc.tensor.matmul(dP, warmS[:, 0:128], warmS, start=True, stop=True)

        def dummy_sc(n=1):
            # sAB-bank filler for INSIDE the oproj drain loop, where an "mm"
            # filler would rotate oP's buffers and break its matmul/copy
            # double-buffering. Only safe once attention is over.
            for _ in range(n):
                dP = ps.tile([128, 2 * QTILE], F32, name="sAB", tag="sc", bufs=2)
                nc.tensor.matmul(
                    dP[:, 0:QTILE], warmS[:, 0:128], warmS, start=True, stop=True
                )

        dummy_mm(N_WARM)
        # closures consult this at call time: the final drain sets it so
        # oproj emits PE filler between its copy-paced matmuls
        dense_drain = [False]

        # --- constants / weights (persistent) ---
        # Ordered so the head of the kernel only waits for wq + the first
        # x chunk: wo (first needed ~40us in) and small consts go later.
        # wq in one 256KB DMA, then the first x tile in 2-kt chunks: each
        # descriptor costs ~600ns to issue, so fewer+bigger beats per-kt
        # interleave; a 256KB chunk feeds two matmuls and the stream stays
        # dense from the first dummy onward (any PE gap resets the activity
        # monitor's ~4.5us promotion clock).
        wq_sb = consts.tile([128, KT, 128], BF16)
        nc.sync.dma_start(out=wq_sb, in_=wq)
        xt00 = xt_pool.tile([128, KT, QTILE], BF16, name="xt", tag="xt")
        for k0, k1 in ((0, 3), (3, 6), (6, 8)):
            nc.sync.dma_start(out=xt00[:, k0:k1, :], in_=xT[:, 0, k0:k1, :])
        wk_sb = consts.tile([128, KT, 128], BF16)
        nc.sync.dma_start(out=wk_sb, in_=wk)
        wv_sb = consts.tile([128, KT, 128], BF16)
        nc.sync.dma_start(out=wv_sb, in_=wv)
        tri_sb = consts.tile([128, 128], BF16)
        nc.sync.dma_start(out=tri_sb, in_=tri)
        ind_sb = consts.tile([65, 128], F32R)
        nc.sync.dma_start(out=ind_sb, in_=ind.bitcast(F32R))
        ident_sb = consts.tile([128, 128], BF16)
        nc.sync.dma_start(out=ident_sb, in_=ident)
        wo_sb = consts.tile([128, KT, 128], F32R)
        nc.sync.dma_start(
            out=wo_sb, in_=wo.rearrange("p (jt m) -> p jt m", jt=KT).bitcast(F32R)
        )

        finishq = []  # [finish_rb(qi), finish_oproj(qi)] of the prev q-tile
        projq = []  # stage-A closures of the NEXT batch

        def build_stage_a(b):
            """Allocate batch b's persistent tiles and return them with the
            list of closures that emit its projection work."""
            tb = b * S
            qT_sb = batch_pool.tile([128, S], BF16, name="qT_sb")
            # kT2 half 0: [kA; 0], half 1: [0; kB] — full-K scores keep the
            # whole PE array active so HAM stays at full clock. Zero halves
            # are initialized on the first visit of each pool buffer (b<2)
            # and inherited afterwards.
            kT2_sb = batch_pool.tile([128, 2, S], BF16, name="kT2_sb")
            vtok_sb = batch_pool.tile([128, NKT, 192], BF16, name="vtok_sb")
            xts = [
                xt00
                if (b == 0 and tt == 0)
                else xt_pool.tile([128, KT, QTILE], BF16, name="xt", tag="xt")
                for tt in range(NQT)
            ]

            issued = set()

            def prefetch(tt, b=b, xts=xts, issued=issued):
                if tt in issued or (b == 0 and tt == 0):
                    return  # only issue each tile's DMA once
                issued.add(tt)
                g = b * NQT + tt
                nc.sync.dma_start(out=xts[tt], in_=xT[:, g, :, :])

            closures = []
            if b < 2:

                def init_consts(kT2_sb=kT2_sb, vtok_sb=vtok_sb):
                    # ones column + kT2 zero halves; GpSimd is otherwise idle
                    nc.gpsimd.memset(vtok_sb[:, :, 64:65], 1.0)
                    nc.gpsimd.memset(kT2_sb[64:128, 0, :], 0.0)
                    nc.gpsimd.memset(kT2_sb[0:64, 1, :], 0.0)

                closures.append(init_consts)

            for tt in range(NQT):
                cell = {}

                def proj_q(tt=tt, qT_sb=qT_sb, xts=xts, cell=cell):
                    xt = xts[tt]
                    cell["xt"] = xt
                    qP = ps.tile([128, QTILE], F32, name="qP", tag="mm", bufs=2)
                    for kt in range(KT):
                        nc.tensor.matmul(
                            qP,
                            wq_sb[:, kt, :],
                            xt[:, kt, :],
                            start=(kt == 0),
                            stop=(kt == KT - 1),
                        )
                    nc.vector.tensor_copy(
                        qT_sb[:, tt * QTILE : (tt + 1) * QTILE], qP
                    )

                def proj_k(tt=tt, kT2_sb=kT2_sb, cell=cell):
                    xt = cell["xt"]
                    kP = ps.tile([128, QTILE], F32, name="kP", tag="mm", bufs=2)
                    for kt in range(KT):
                        nc.tensor.matmul(
                            kP,
                            wk_sb[:, kt, :],
                            xt[:, kt, :],
                            start=(kt == 0),
                            stop=(kt == KT - 1),
                        )
                    nc.vector.tensor_copy(
                        kT2_sb[0:64, 0, tt * QTILE : (tt + 1) * QTILE], kP[0:64, :]
                    )
                    nc.vector.tensor_copy(
                        kT2_sb[64:128, 1, tt * QTILE : (tt + 1) * QTILE],
                        kP[64:128, :],
                    )

                def proj_v(tt=tt, prefetch=prefetch, cell=cell):
                    xt = cell["xt"]
                    vP = ps.tile([128, QTILE], F32, name="vP", tag="mm", bufs=2)
                    for kt in range(KT):
                        nc.tensor.matmul(
                            vP,
                            wv_sb[:, kt, :],
                            xt[:, kt, :],
                            start=(kt == 0),
                            stop=(kt == KT - 1),
                        )
                    vT_tmp = vtmp_pool.tile([128, QTILE], BF16, name="vT_tmp")
                    nc.vector.tensor_copy(vT_tmp, vP)
                    cell["vT"] = vT_tmp
                    if tt + 3 < NQT:
                        # xt buffer for tt is free after this group; refill
                        prefetch(tt + 3)

                def vtrans(tt=tt, vtok_sb=vtok_sb, cell=cell):
                    vT_tmp = cell["vT"]
                    for s in range(QTILE // 128):
                        vtokP = ps.tile(
                            [128, 128], BF16, name="vtokP", tag="mm", bufs=2
                        )
                        nc.tensor.transpose(
                            vtokP, vT_tmp[:, s * 128 : (s + 1) * 128], ident_sb
                        )
                        m = tt * 4 + s
                        nc.vector.tensor_copy(vtok_sb[:, m, 0:64], vtokP[:, 0:64])
                        nc.vector.tensor_copy(
                            vtok_sb[:, m, 128:192], vtokP[:, 64:128]
                        )

                closures += [proj_q, proj_k, proj_v, vtrans]
            return (qT_sb, kT2_sb, vtok_sb), prefetch, closures

        def attention(b, tiles):
            qT_sb, kT2_sb, vtok_sb = tiles
            tb = b * S
            for qi in range(NQT):
                q0 = qi * QTILE  # batch-local q base
                nk = 4 * qi + 4  # k-tiles for this q-tile (block-causal)
                ctxAB = ps.tile(
                    [128, 2 * QTILE], F32, name="ctxAB", tag="ctx", bufs=1
                )

                def geom(m, qi=qi):
                    d_off = m - 4 * qi
                    if d_off >= 0:
                        return QTILE - 128 * d_off, 128 * d_off, True
                    return QTILE, 0, False

                # finish pops: rb early (i=2), oproj spread so the last half
                # lands in the LAG-flush slots where the PE would otherwise
                # idle behind exp-gated ctx matmuls; projq keeps 4 groups in
                # reserve for the same flush region (by tile end the exp
                # stream lags the scores, so every flush ctx waits ~1us).
                finish_slots = sorted({2, 5, nk - 1, nk + 1})
                exps = {}
                for i in range(nk + LAG):
                    popped = False
                    if i in finish_slots and finishq:
                        finishq.pop(0)()
                        popped = True
                    elif (
                        projq
                        and (i % 2 == 1 or i >= 10 or i >= nk)
                        and (len(projq) > 4 or i >= nk - 3)
                    ):
                        projq.pop(0)()
                        popped = True
                    if not popped and i >= nk - 3:
                        dummy_mm(1)
                    if i < nk:
                        m = i
                        width, qoff, diag = geom(m)
                        sAB = ps.tile(
                            [128, 2 * QTILE], F32, name="sAB", tag="sc", bufs=2
                        )
                        nc.tensor.matmul(
                            sAB[:, 0:width],
                            kT2_sb[:, 0, m * 128 : (m + 1) * 128],
                            qT_sb[:, q0 + qoff : q0 + QTILE],
                            start=True,
                            stop=True,
                        )
                        nc.tensor.matmul(
                            sAB[:, QTILE : QTILE + width],
                            kT2_sb[:, 1, m * 128 : (m + 1) * 128],
                            qT_sb[:, q0 + qoff : q0 + QTILE],
                            start=True,
                            stop=True,
                        )
                        # one wide exp covers both heads; the gap region
                        # [width:QTILE] holds unread junk for diag tiles.
                        eAB = exp_pool.tile(
                            [128, 2 * QTILE], BF16, name="eAB", tag="exp"
                        )
                        nc.scalar.activation(
                            eAB[:, 0 : QTILE + width],
                            sAB[:, 0 : QTILE + width],
                            EXP_FUNC,
                            scale=INV_SQRT_DK,
                        )
                        if diag:
                            nc.vector.tensor_mul(eAB[:, 0:128], eAB[:, 0:128], tri_sb)
                            nc.vector.tensor_mul(
                                eAB[:, QTILE : QTILE + 128],
                                eAB[:, QTILE : QTILE + 128],
                                tri_sb,
                            )
                        exps[m] = eAB

                    j = i - LAG
                    if j >= 0:
                        width, qoff, _ = geom(j)
                        first = j == 0
                        last = j == nk - 1
                        eAB = exps.pop(j)
                        nc.tensor.matmul(
                            ctxAB[:, qoff:QTILE],
                            vtok_sb[:, j, 0:128],
                            eAB[:, 0:width],
                            start=first,
                            stop=last,
                            skip_group_check=True,
                        )
                        nc.tensor.matmul(
                            ctxAB[:, QTILE + qoff : 2 * QTILE],
                            vtok_sb[:, j, 64:192],
                            eAB[:, QTILE : QTILE + width],
                            start=first,
                            stop=last,
                            skip_group_check=True,
                        )

                # normalization part 1 (immediate, frees the ctx PSUM slots):
                # sumB is already on partition 0 of the B half (shared ones
                # col) and sumA on partition 64 of the A half — both are
                # same-partition scalar copies into s2; rows 1:63 of s2 hold
                # zeros (memset once per pool buffer) so the K=65 broadcast
                # matmul ignores the junk. ctx copies go on the vector
                # engine so the scalar EXP stream is not delayed.
                s2 = small_pool.tile([65, 2, QTILE], F32R, name="s2")
                if b == 0 and qi < 3:
                    nc.gpsimd.memset(s2[:, 0, :].bitcast(F32), 0.0)
                nc.vector.tensor_copy(s2[0:1, 0, :], ctxAB[0:1, QTILE : 2 * QTILE])
                nc.vector.tensor_copy(s2[64:65, 0, :], ctxAB[64:65, 0:QTILE])
                ctxn = ctxn_pool.tile([128, QTILE], F32R, name="ctxn")
                nc.vector.tensor_copy(ctxn[0:64, :], ctxAB[0:64, 0:QTILE])
                nc.vector.tensor_copy(
                    ctxn[64:128, :], ctxAB[64:128, QTILE : 2 * QTILE]
                )

                def finish_rb(qi=qi, ctxn=ctxn, s2=s2):
                    # deferred stage 1: K=65 indicator matmul broadcasts both
                    # sums to [128, QTILE] PSUM (row 0 = sumB -> partitions
                    # 64:128, row 64 = sumA -> 0:64); fast-approx reciprocal
                    # + normalize.
                    rbP = ps.tile([128, QTILE], F32, name="rbP", tag="mm", bufs=2)
                    nc.tensor.matmul(
                        rbP, ind_sb, s2[0:65, 0, :], start=True, stop=True
                    )
                    rb_sb = small_pool.tile([128, QTILE], F32, name="rb_sb")
                    nc.vector.reciprocal_approx_fast(out=rb_sb, in_=rbP)
                    nc.vector.tensor_mul(ctxn, ctxn, rb_sb.bitcast(F32R))

                cell = {}

                def oproj_half(h, qi=qi, q0=q0, tb=tb, ctxn=ctxn, cell=cell):
                    # deferred stage 2: output projection (ctxn is normalized
                    # by the time this runs several m-iterations later), in
                    # two pop-halves so attention matmuls interleave with the
                    # copy-paced PSUM drain.
                    if h == 0:
                        cell["o_all"] = oall_pool.tile(
                            [128, KT, QTILE], FP16, name="o_all"
                        )
                    o_all = cell["o_all"]
                    jt0, jt1 = h * (KT // 2), (h + 1) * (KT // 2)
                    g = (tb + q0) // QTILE
                    odst = outT[:, g, jt0:jt1, :]
                    engs = [nc.scalar.copy, nc.vector.tensor_copy]
                    for jt in range(jt0, jt1):
                        oP = ps.tile([128, QTILE], F32, name="oP", tag="mm", bufs=2)
                        nc.tensor.matmul(
                            oP, wo_sb[:, jt, :], ctxn, start=True, stop=True
                        )
                        engs[jt % 2](o_all[:, jt, :], oP)
                        if dense_drain[0]:
                            # in the final drain these matmuls pace at the
                            # copy cadence (~50% PE); filler between them
                            # keeps the activity monitor at full rate
                            dummy_sc(1)
                    # per-half DMA: halves the final drain's trailing transfer
                    nc.sync.dma_start(out=odst, in_=o_all[:, jt0:jt1, :])

                finishq.extend(
                    [
                        finish_rb,
                        lambda f=oproj_half: f(0),
                        lambda f=oproj_half: f(1),
                    ]
                )

        # batch 0's stage A runs inline (nothing to overlap with); its xt
        # prefetches ride the pool rotation as the closures execute.
        tiles0, prefetch0, cl0 = build_stage_a(0)
        for tt in range(1, NQT):
            prefetch0(tt)
        for c in cl0:
            c()
        tiles = tiles0
        for b in range(B):
            if b + 1 < B:
                next_tiles, next_prefetch, next_cl = build_stage_a(b + 1)
                # keep the xt pipeline 3 tiles deep across the batch seam
                for tt in range(min(3, NQT)):
                    next_prefetch(tt)
                projq.extend(next_cl)
            attention(b, tiles)
            while projq:
                projq.pop(0)()
            if b + 1 < B:
                tiles = next_tiles

        # final drain: the copy/recip chains leave the PE ~40% idle, which
        # trips the activity monitor into half-rate mode and stretches the
        # remaining matmuls 2x. Garbage matmuls keep it hot; they go BEFORE
        # each pop because the PE queue is in-order — filler emitted after a
        # stalled real matmul can never run during that stall.
        dense_drain[0] = True
        while finishq:
            dummy_mm(3)
            finishq.pop(0)()
        dummy_mm(2)

    nc.compile()
    return nc


_NC = None


def _get_nc():
    global _NC
    if _NC is None:
        _NC = build_nc()
    return _NC


def make_in_maps(x, W_Q, W_K, W_V, W_O):
    import ml_dtypes

    NT = T // QTILE
    # x and the q/k/v weights ship as bf16: halves the 32MB/core input DMA
    # (q/k/v are consumed as bf16 downstream anyway; verified rel err 2.6e-3
    # vs the 2e-2 gate). W_O stays fp32 for the fp32r output-projection path.
    # Layouts are pre-tiled on the host so each DMA reads 2-8KB contiguous
    # runs per partition: x -> [p, tile, kt, t], w -> [p, kt, m].
    xT = np.asarray(x, dtype=np.float32).reshape(T, D).T.astype(ml_dtypes.bfloat16)
    xTh = np.ascontiguousarray(
        xT.reshape(KT, 128, NT, QTILE).transpose(1, 2, 0, 3)
    )
    W_Q = np.asarray(W_Q, dtype=np.float32).astype(ml_dtypes.bfloat16)
    W_K = np.asarray(W_K, dtype=np.float32).astype(ml_dtypes.bfloat16)
    W_V = np.asarray(W_V, dtype=np.float32).astype(ml_dtypes.bfloat16)
    W_O = np.asarray(W_O, dtype=np.float32)

    def wtile(W, sl):
        # [p, kt, m] with m contiguous per (p, kt): W[sl].T is [D, 128]
        return np.ascontiguousarray(
            W[sl, :].T.reshape(KT, 128, 128).transpose(1, 0, 2)
        )

    tri = np.triu(np.ones((128, 128), dtype=ml_dtypes.bfloat16))  # tri[k,q]=1 iff q>=k
    ind2 = np.zeros((65, 128), dtype=np.float32)
    ind2[0, 64:128] = 1.0  # row 0 = sumB -> partitions 64:128
    ind2[64, 0:64] = 1.0  # row 64 = sumA -> partitions 0:64
    ident = np.eye(128, dtype=ml_dtypes.bfloat16)
    in_maps = []
    for c in range(NCORES):
        sl = slice(c * 128, (c + 1) * 128)
        in_maps.append(
            {
                "xT": xTh,
                "wq": wtile(W_Q, sl),
                "wk": wtile(W_K, sl),
                "wv": wtile(W_V, sl),
                "wo": np.ascontiguousarray(W_O.T[sl, :]),
                "tri": tri,
                "ind": ind2,
                "ident": ident,
            }
        )
    return in_maps


def kernel(x, W_Q, W_K, W_V, W_O, _results_hook=None):
    nc = _get_nc()
    in_maps = make_in_maps(x, W_Q, W_K, W_V, W_O)
    res = run_bass_kernel_spmd(nc, in_maps, list(range(NCORES)))
    if _results_hook is not None:
        _results_hook(res)
    NT = T // QTILE
    acc = np.zeros((128, NT, KT, QTILE), dtype=np.float64)
    for c in range(NCORES):
        acc += res.results[c]["outT"]
    # [p, g, jt, t] -> outT[jt*128+p, g*512+t] -> [T, D] -> [B, S, D]
    outT = acc.transpose(2, 0, 1, 3).reshape(D, T)
    out = np.ascontiguousarray(outT.T).reshape(B, S, D).astype(np.float32)
    return out



# revision 46
# speedup vs baseline: 1.1836x; 1.0061x over previous
"""Causal multi-head self-attention on 8 TRN2 NeuronCores.

Sharding: tensor-parallel over heads. 16 heads / 8 cores = 2 heads per core.
Each core computes q/k/v projections for its 2 heads (feature-major via
fp32r matmuls), block-causal attention (scores kept k-major so softmax sums
come from a fused ones-column in the attn@v matmul and no transposes are
needed), and a partial output projection against its 128-column slice of
W_O. The host sums the 8 partial outputs.

Layouts on core c (heads 2c, 2c+1 = "A", "B"):
  qT/kT  [128, 2048]  feature-major; rows 0:64 head A dk, 64:128 head B
  vtok   [128, 16, 192] token-major v (PE-transposed): cols 0:64 vA, col 64
         a shared ones column, cols 128:192 vB. Head A's ctx lhsT slice is
         cols 0:128 (ctxA on partitions 0:64, sumA on 64); head B's slice is
         cols 64:192, so the same ones column becomes its col 0 (sumB on
         partition 0) and vB lands on partitions 64:128 — no partition-shift
         DMA is needed to assemble ctxn.
  scoresT[128 k-tok, <=512 q-tok] per (q-tile, k-tile); exp'd on ScalarE
  ctxAB  [128, 2*512] PSUM; A half rows 0:64 + sum row 64, B half sum row 0
         + rows 64:128
  out    partial [1024, 8192] feature-major; host sums over cores + transposes

Pipelining: the whole kernel is one deferred-work machine. Attention of
batch b interleaves (as pop-closures in its k-tile loop) the q/k/v
projections + v-transposes of batch b+1, plus the softmax-normalization
(finish_rb) and output-projection (finish_oproj) of the previous q-tile.
This keeps the PE stream dense (pstate stays at max clock) and gives the
scalar engine's exp stream (~1us per k-iter, the attention pacer) slack.

Softmax denominators: sumA sits on partition 64, sumB on partition 0 of the
two ctx halves; both are same-partition scalar copies into s2, whose rows
1:63 hold persistent zeros, and a K=65 indicator matmul broadcasts them to
[128, QTILE]. reciprocal_approx_fast (~18 bits) + one DVE multiply
normalize ctx.

Toolchain constraints honored here: col-offset tile_position is illegal for
4-byte matmul dtypes; fp32r consumers need fp32r-typed producers; x is
transposed on the host so every DMA has a contiguous innermost run (>=2KB);
fp32r matmuls with moving dim >= 256 run at full PE rate.
"""

import numpy as np
from contextlib import ExitStack

import concourse.bass as bass
import concourse.tile as tile
from concourse import bacc, mybir
from concourse.bass_utils import run_bass_kernel_spmd

F32 = mybir.dt.float32
F32R = mybir.dt.float32r
BF16 = mybir.dt.bfloat16
FP16 = mybir.dt.float16

B, S, D, H = 4, 2048, 1024, 16
DK = D // H  # 64
NCORES = 8
T = B * S  # 8192 tokens
KT = D // 128  # 8 contraction tiles for projections
QTILE = 512  # q-tile width (tokens)
KTILE = 128  # k-tile width (tokens)
NQT = S // QTILE  # 4 q-tiles per batch
NKT = S // KTILE  # 16 k-tiles per batch
LAG = 2  # ctx matmuls trail scores by this many k-tiles
N_WARM = 7  # PE warm-up garbage matmuls at kernel start
EXP_FUNC = mybir.ActivationFunctionType.Exp
INV_SQRT_DK = 1.0 / np.sqrt(DK)


def build_nc():
    nc = bacc.Bacc("TRN2", target_bir_lowering=False, debug=False)

    # host pre-layouts so every DMA runs long-contiguous per partition:
    # x [128, 16 tiles, KT, 512] -> 8KB runs; weights [128, KT, 128] -> 2KB;
    # output [128, 16, KT, 512] -> 4KB-per-half runs (host inverts).
    NT = T // QTILE  # 16 global q-tiles
    xT = nc.dram_tensor("xT", [128, NT, KT, QTILE], BF16, kind="ExternalInput").ap()
    wq = nc.dram_tensor("wq", [128, KT, 128], BF16, kind="ExternalInput").ap()
    wk = nc.dram_tensor("wk", [128, KT, 128], BF16, kind="ExternalInput").ap()
    wv = nc.dram_tensor("wv", [128, KT, 128], BF16, kind="ExternalInput").ap()
    wo = nc.dram_tensor("wo", [128, D], F32, kind="ExternalInput").ap()
    tri = nc.dram_tensor("tri", [128, 128], BF16, kind="ExternalInput").ap()
    ind = nc.dram_tensor("ind", [65, 128], F32, kind="ExternalInput").ap()
    ident = nc.dram_tensor("ident", [128, 128], BF16, kind="ExternalInput").ap()
    outT = nc.dram_tensor("outT", [128, NT, KT, QTILE], FP16, kind="ExternalOutput").ap()

    with ExitStack() as ctx:
        tc = ctx.enter_context(tile.TileContext(nc))
        consts = ctx.enter_context(tc.tile_pool(name="consts", bufs=1))
        xt_pool = ctx.enter_context(tc.tile_pool(name="xt_pool", bufs=3))
        batch_pool = ctx.enter_context(tc.tile_pool(name="batch_pool", bufs=2))
        vtmp_pool = ctx.enter_context(tc.tile_pool(name="vtmp_pool", bufs=3))
        exp_pool = ctx.enter_context(tc.tile_pool(name="exp_pool", bufs=5))
        ctxn_pool = ctx.enter_context(tc.tile_pool(name="ctxn_pool", bufs=3))
        oall_pool = ctx.enter_context(tc.tile_pool(name="oall_pool", bufs=2))
        small_pool = ctx.enter_context(tc.tile_pool(name="small_pool", bufs=3))
        ps = ctx.enter_context(tc.tile_pool(name="ps", bufs=1, space="PSUM"))

        # --- PE warm-up ---
        # The HW activity monitor starts the PE in a half-rate state and only
        # promotes after sustained matmul activity; garbage matmuls during the
        # initial DMA wait start that clock early so the real projections run
        # at full rate sooner.
        warmS = consts.tile([128, QTILE], F32R, name="warmS")
        nc.gpsimd.memset(warmS.bitcast(F32), 1.0)

        def dummy_mm(n=1):
            # tag "mm": those buffers recycle fast (their consumers are plain
            # copies), so a filler never parks the in-order PE queue behind a
            # slow consumer the way an sAB buffer (exp-gated) would.
            for _ in range(n):
                dP = ps.tile([128, QTILE], F32, name="oP", tag="mm", bufs=2)
                nc.tensor.matmul(dP, warmS[:, 0:128], warmS, start=True, stop=True)

        def dummy_sc(n=1):
            # sAB-bank filler for INSIDE the oproj drain loop, where an "mm"
            # filler would rotate oP's buffers and break its matmul/copy
            # double-buffering. Only safe once attention is over.
            for _ in range(n):
                dP = ps.tile([128, 2 * QTILE], F32, name="sAB", tag="sc", bufs=2)
                nc.tensor.matmul(
                    dP[:, 0:QTILE], warmS[:, 0:128], warmS, start=True, stop=True
                )

        dummy_mm(N_WARM)
        # closures consult this at call time: the final drain sets it so
        # oproj emits PE filler between its copy-paced matmuls
        dense_drain = [False]

        # --- constants / weights (persistent) ---
        # Ordered so the head of the kernel only waits for wq + the first
        # x chunk: wo (first needed ~40us in) and small consts go later.
        # wq in one 256KB DMA, then the first x tile in 2-kt chunks: each
        # descriptor costs ~600ns to issue, so fewer+bigger beats per-kt
        # interleave; a 256KB chunk feeds two matmuls and the stream stays
        # dense from the first dummy onward (any PE gap resets the activity
        # monitor's ~4.5us promotion clock).
        wq_sb = consts.tile([128, KT, 128], BF16)
        nc.sync.dma_start(out=wq_sb, in_=wq)
        xt00 = xt_pool.tile([128, KT, QTILE], BF16, name="xt", tag="xt")
        for k0, k1 in ((0, 3), (3, 6), (6, 8)):
            nc.sync.dma_start(out=xt00[:, k0:k1, :], in_=xT[:, 0, k0:k1, :])
        wk_sb = consts.tile([128, KT, 128], BF16)
        nc.sync.dma_start(out=wk_sb, in_=wk)
        wv_sb = consts.tile([128, KT, 128], BF16)
        nc.sync.dma_start(out=wv_sb, in_=wv)
        tri_sb = consts.tile([128, 128], BF16)
        nc.sync.dma_start(out=tri_sb, in_=tri)
        ind_sb = consts.tile([65, 128], F32R)
        nc.sync.dma_start(out=ind_sb, in_=ind.bitcast(F32R))
        ident_sb = consts.tile([128, 128], BF16)
        nc.sync.dma_start(out=ident_sb, in_=ident)
        wo_sb = consts.tile([128, KT, 128], F32R)
        nc.sync.dma_start(
            out=wo_sb, in_=wo.rearrange("p (jt m) -> p jt m", jt=KT).bitcast(F32R)
        )

        finishq = []  # [finish_rb(qi), finish_oproj(qi)] of the prev q-tile
        projq = []  # stage-A closures of the NEXT batch

        def build_stage_a(b):
            """Allocate batch b's persistent tiles and return them with the
            list of closures that emit its projection work."""
            tb = b * S
            qT_sb = batch_pool.tile([128, S], BF16, name="qT_sb")
            # kT2 half 0: [kA; 0], half 1: [0; kB] — full-K scores keep the
            # whole PE array active so HAM stays at full clock. Zero halves
            # are initialized on the first visit of each pool buffer (b<2)
            # and inherited afterwards.
            kT2_sb = batch_pool.tile([128, 2, S], BF16, name="kT2_sb")
            vtok_sb = batch_pool.tile([128, NKT, 192], BF16, name="vtok_sb")
            xts = [
                xt00
                if (b == 0 and tt == 0)
                else xt_pool.tile([128, KT, QTILE], BF16, name="xt", tag="xt")
                for tt in range(NQT)
            ]

            issued = set()

            def prefetch(tt, b=b, xts=xts, issued=issued):
                if tt in issued or (b == 0 and tt == 0):
                    return  # only issue each tile's DMA once
                issued.add(tt)
                g = b * NQT + tt
                nc.sync.dma_start(out=xts[tt], in_=xT[:, g, :, :])

            closures = []
            if b < 2:

                def init_consts(kT2_sb=kT2_sb, vtok_sb=vtok_sb):
                    # ones column + kT2 zero halves; GpSimd is otherwise idle
                    nc.gpsimd.memset(vtok_sb[:, :, 64:65], 1.0)
                    nc.gpsimd.memset(kT2_sb[64:128, 0, :], 0.0)
                    nc.gpsimd.memset(kT2_sb[0:64, 1, :], 0.0)

                closures.append(init_consts)

            for tt in range(NQT):
                cell = {}

                def proj_q(tt=tt, qT_sb=qT_sb, xts=xts, cell=cell):
                    xt = xts[tt]
                    cell["xt"] = xt
                    qP = ps.tile([128, QTILE], F32, name="qP", tag="mm", bufs=2)
                    for kt in range(KT):
                        nc.tensor.matmul(
                            qP,
                            wq_sb[:, kt, :],
                            xt[:, kt, :],
                            start=(kt == 0),
                            stop=(kt == KT - 1),
                        )
                    nc.vector.tensor_copy(
                        qT_sb[:, tt * QTILE : (tt + 1) * QTILE], qP
                    )

                def proj_k(tt=tt, kT2_sb=kT2_sb, cell=cell):
                    xt = cell["xt"]
                    kP = ps.tile([128, QTILE], F32, name="kP", tag="mm", bufs=2)
                    for kt in range(KT):
                        nc.tensor.matmul(
                            kP,
                            wk_sb[:, kt, :],
                            xt[:, kt, :],
                            start=(kt == 0),
                            stop=(kt == KT - 1),
                        )
                    nc.vector.tensor_copy(
                        kT2_sb[0:64, 0, tt * QTILE : (tt + 1) * QTILE], kP[0:64, :]
                    )
                    nc.vector.tensor_copy(
                        kT2_sb[64:128, 1, tt * QTILE : (tt + 1) * QTILE],
                        kP[64:128, :],
                    )

                def proj_v(tt=tt, prefetch=prefetch, cell=cell):
                    xt = cell["xt"]
                    vP = ps.tile([128, QTILE], F32, name="vP", tag="mm", bufs=2)
                    for kt in range(KT):
                        nc.tensor.matmul(
                            vP,
                            wv_sb[:, kt, :],
                            xt[:, kt, :],
                            start=(kt == 0),
                            stop=(kt == KT - 1),
                        )
                    vT_tmp = vtmp_pool.tile([128, QTILE], BF16, name="vT_tmp")
                    nc.vector.tensor_copy(vT_tmp, vP)
                    cell["vT"] = vT_tmp
                    if tt + 3 < NQT:
                        # xt buffer for tt is free after this group; refill
                        prefetch(tt + 3)

                def vtrans(tt=tt, vtok_sb=vtok_sb, cell=cell):
                    vT_tmp = cell["vT"]
                    for s in range(QTILE // 128):
                        vtokP = ps.tile(
                            [128, 128], BF16, name="vtokP", tag="mm", bufs=2
                        )
                        nc.tensor.transpose(
                            vtokP, vT_tmp[:, s * 128 : (s + 1) * 128], ident_sb
                        )
                        m = tt * 4 + s
                        nc.vector.tensor_copy(vtok_sb[:, m, 0:64], vtokP[:, 0:64])
                        nc.vector.tensor_copy(
                            vtok_sb[:, m, 128:192], vtokP[:, 64:128]
                        )

                closures += [proj_q, proj_k, proj_v, vtrans]
            return (qT_sb, kT2_sb, vtok_sb), prefetch, closures

        def attention(b, tiles):
            qT_sb, kT2_sb, vtok_sb = tiles
            tb = b * S
            for qi in range(NQT):
                q0 = qi * QTILE  # batch-local q base
                nk = 4 * qi + 4  # k-tiles for this q-tile (block-causal)
                ctxAB = ps.tile(
                    [128, 2 * QTILE], F32, name="ctxAB", tag="ctx", bufs=1
                )

                def geom(m, qi=qi):
                    d_off = m - 4 * qi
                    if d_off >= 0:
                        return QTILE - 128 * d_off, 128 * d_off, True
                    return QTILE, 0, False

                # finish pops: rb early (i=2), oproj spread so the last half
                # lands in the LAG-flush slots where the PE would otherwise
                # idle behind exp-gated ctx matmuls; projq keeps 4 groups in
                # reserve for the same flush region (by tile end the exp
                # stream lags the scores, so every flush ctx waits ~1us).
                finish_slots = sorted({2, 5, nk - 1, nk + 1})
                exps = {}
                for i in range(nk + LAG):
                    popped = False
                    if i in finish_slots and finishq:
                        finishq.pop(0)()
                        popped = True
                    elif (
                        projq
                        and (i % 2 == 1 or i >= 10 or i >= nk)
                        and (len(projq) > 4 or i >= nk - 3)
                    ):
                        projq.pop(0)()
                        popped = True
                    if not popped and i >= nk - 3:
                        dummy_mm(1)
                    if i < nk:
                        m = i
                        width, qoff, diag = geom(m)
                        sAB = ps.tile(
                            [128, 2 * QTILE], F32, name="sAB", tag="sc", bufs=2
                        )
                        nc.tensor.matmul(
                            sAB[:, 0:width],
                            kT2_sb[:, 0, m * 128 : (m + 1) * 128],
                            qT_sb[:, q0 + qoff : q0 + QTILE],
                            start=True,
                            stop=True,
                        )
                        nc.tensor.matmul(
                            sAB[:, QTILE : QTILE + width],
                            kT2_sb[:, 1, m * 128 : (m + 1) * 128],
                            qT_sb[:, q0 + qoff : q0 + QTILE],
                            start=True,
                            stop=True,
                        )
                        # one wide exp covers both heads; the gap region
                        # [width:QTILE] holds unread junk for diag tiles.
                        eAB = exp_pool.tile(
                            [128, 2 * QTILE], BF16, name="eAB", tag="exp"
                        )
                        nc.scalar.activation(
                            eAB[:, 0 : QTILE + width],
                            sAB[:, 0 : QTILE + width],
                            EXP_FUNC,
                            scale=INV_SQRT_DK,
                        )
                        if diag:
                            nc.vector.tensor_mul(eAB[:, 0:128], eAB[:, 0:128], tri_sb)
                            nc.vector.tensor_mul(
                                eAB[:, QTILE : QTILE + 128],
                                eAB[:, QTILE : QTILE + 128],
                                tri_sb,
                            )
                        exps[m] = eAB

                    j = i - LAG
                    if j >= 0:
                        width, qoff, _ = geom(j)
                        first = j == 0
                        last = j == nk - 1
                        eAB = exps.pop(j)
                        nc.tensor.matmul(
                            ctxAB[:, qoff:QTILE],
                            vtok_sb[:, j, 0:128],
                            eAB[:, 0:width],
                            start=first,
                            stop=last,
                            skip_group_check=True,
                        )
                        nc.tensor.matmul(
                            ctxAB[:, QTILE + qoff : 2 * QTILE],
                            vtok_sb[:, j, 64:192],
                            eAB[:, QTILE : QTILE + width],
                            start=first,
                            stop=last,
                            skip_group_check=True,
                        )

                # normalization part 1 (immediate, frees the ctx PSUM slots):
                # sumB is already on partition 0 of the B half (shared ones
                # col) and sumA on partition 64 of the A half — both are
                # same-partition scalar copies into s2; rows 1:63 of s2 hold
                # zeros (memset once per pool buffer) so the K=65 broadcast
                # matmul ignores the junk. ctx copies go on the vector
                # engine so the scalar EXP stream is not delayed.
                s2 = small_pool.tile([65, 2, QTILE], F32R, name="s2")
                if b == 0 and qi < 3:
                    nc.gpsimd.memset(s2[:, 0, :].bitcast(F32), 0.0)
                nc.vector.tensor_copy(s2[0:1, 0, :], ctxAB[0:1, QTILE : 2 * QTILE])
                nc.vector.tensor_copy(s2[64:65, 0, :], ctxAB[64:65, 0:QTILE])
                ctxn = ctxn_pool.tile([128, QTILE], F32R, name="ctxn")
                nc.vector.tensor_copy(ctxn[0:64, :], ctxAB[0:64, 0:QTILE])
                nc.vector.tensor_copy(
                    ctxn[64:128, :], ctxAB[64:128, QTILE : 2 * QTILE]
                )

                def finish_rb(qi=qi, ctxn=ctxn, s2=s2):
                    # deferred stage 1: K=65 indicator matmul broadcasts both
                    # sums to [128, QTILE] PSUM (row 0 = sumB -> partitions
                    # 64:128, row 64 = sumA -> 0:64); fast-approx reciprocal
                    # + normalize.
                    rbP = ps.tile([128, QTILE], F32, name="rbP", tag="mm", bufs=2)
                    nc.tensor.matmul(
                        rbP, ind_sb, s2[0:65, 0, :], start=True, stop=True
                    )
                    rb_sb = small_pool.tile([128, QTILE], F32, name="rb_sb")
                    nc.vector.reciprocal_approx_fast(out=rb_sb, in_=rbP)
                    nc.vector.tensor_mul(ctxn, ctxn, rb_sb.bitcast(F32R))

                cell = {}

                def oproj_half(h, qi=qi, q0=q0, tb=tb, ctxn=ctxn, cell=cell):
                    # deferred stage 2: output projection (ctxn is normalized
                    # by the time this runs several m-iterations later), in
                    # two pop-halves so attention matmuls interleave with the
                    # copy-paced PSUM drain.
                    if h == 0:
                        cell["o_all"] = oall_pool.tile(
                            [128, KT, QTILE], FP16, name="o_all"
                        )
                    o_all = cell["o_all"]
                    jt0, jt1 = h * (KT // 2), (h + 1) * (KT // 2)
                    g = (tb + q0) // QTILE
                    odst = outT[:, g, jt0:jt1, :]
                    engs = [nc.scalar.copy, nc.vector.tensor_copy]
                    for jt in range(jt0, jt1):
                        oP = ps.tile([128, QTILE], F32, name="oP", tag="mm", bufs=2)
                        nc.tensor.matmul(
                            oP, wo_sb[:, jt, :], ctxn, start=True, stop=True
                        )
                        engs[jt % 2](o_all[:, jt, :], oP)
                        if dense_drain[0]:
                            # in the final drain these matmuls pace at the
                            # copy cadence (~50% PE); filler between them
                            # keeps the activity monitor at full rate
                            dummy_sc(1)
                    # per-half DMA: halves the final drain's trailing transfer
                    nc.sync.dma_start(out=odst, in_=o_all[:, jt0:jt1, :])

                finishq.extend(
                    [
                        finish_rb,
                        lambda f=oproj_half: f(0),
                        lambda f=oproj_half: f(1),
                    ]
                )

        # batch 0's stage A runs inline (nothing to overlap with); its xt
        # prefetches ride the pool rotation as the closures execute.
        tiles0, prefetch0, cl0 = build_stage_a(0)
        for tt in range(1, NQT):
            prefetch0(tt)
        for c in cl0:
            c()
        tiles = tiles0
        for b in range(B):
            if b + 1 < B:
                next_tiles, next_prefetch, next_cl = build_stage_a(b + 1)
                # keep the xt pipeline 3 tiles deep across the batch seam
                for tt in range(min(3, NQT)):
                    next_prefetch(tt)
                projq.extend(next_cl)
            attention(b, tiles)
            while projq:
                projq.pop(0)()
            if b + 1 < B:
                tiles = next_tiles

        # final drain: the copy/recip chains leave the PE ~40% idle, which
        # trips the activity monitor into half-rate mode and stretches the
        # remaining matmuls 2x. Garbage matmuls keep it hot; they go BEFORE
        # each pop because the PE queue is in-order — filler emitted after a
        # stalled real matmul can never run during that stall.
        dense_drain[0] = True
        while finishq:
            dummy_mm(3)
            finishq.pop(0)()
        dummy_mm(2)

    nc.compile()
    return nc


_NC = None


def _get_nc():
    global _NC
    if _NC is None:
        _NC = build_nc()
    return _NC


def make_in_maps(x, W_Q, W_K, W_V, W_O):
    import ml_dtypes

    NT = T // QTILE
    # x and the q/k/v weights ship as bf16: halves the 32MB/core input DMA
    # (q/k/v are consumed as bf16 downstream anyway; verified rel err 2.6e-3
    # vs the 2e-2 gate). W_O stays fp32 for the fp32r output-projection path.
    # Layouts are pre-tiled on the host so each DMA reads 2-8KB contiguous
    # runs per partition: x -> [p, tile, kt, t], w -> [p, kt, m].
    xT = np.asarray(x, dtype=np.float32).reshape(T, D).T.astype(ml_dtypes.bfloat16)
    xTh = np.ascontiguousarray(
        xT.reshape(KT, 128, NT, QTILE).transpose(1, 2, 0, 3)
    )
    W_Q = np.asarray(W_Q, dtype=np.float32).astype(ml_dtypes.bfloat16)
    W_K = np.asarray(W_K, dtype=np.float32).astype(ml_dtypes.bfloat16)
    W_V = np.asarray(W_V, dtype=np.float32).astype(ml_dtypes.bfloat16)
    W_O = np.asarray(W_O, dtype=np.float32)

    def wtile(W, sl):
        # [p, kt, m] with m contiguous per (p, kt): W[sl].T is [D, 128]
        return np.ascontiguousarray(
            W[sl, :].T.reshape(KT, 128, 128).transpose(1, 0, 2)
        )

    tri = np.triu(np.ones((128, 128), dtype=ml_dtypes.bfloat16))  # tri[k,q]=1 iff q>=k
    ind2 = np.zeros((65, 128), dtype=np.float32)
    ind2[0, 64:128] = 1.0  # row 0 = sumB -> partitions 64:128
    ind2[64, 0:64] = 1.0  # row 64 = sumA -> partitions 0:64
    ident = np.eye(128, dtype=ml_dtypes.bfloat16)
    in_maps = []
    for c in range(NCORES):
        sl = slice(c * 128, (c + 1) * 128)
        in_maps.append(
            {
                "xT": xTh,
                "wq": wtile(W_Q, sl),
                "wk": wtile(W_K, sl),
                "wv": wtile(W_V, sl),
                "wo": np.ascontiguousarray(W_O.T[sl, :]),
                "tri": tri,
                "ind": ind2,
                "ident": ident,
            }
        )
    return in_maps


def kernel(x, W_Q, W_K, W_V, W_O, _results_hook=None):
    nc = _get_nc()
    in_maps = make_in_maps(x, W_Q, W_K, W_V, W_O)
    res = run_bass_kernel_spmd(nc, in_maps, list(range(NCORES)))
    if _results_hook is not None:
        _results_hook(res)
    NT = T // QTILE
    acc = np.zeros((128, NT, KT, QTILE), dtype=np.float64)
    for c in range(NCORES):
        acc += res.results[c]["outT"]
    # [p, g, jt, t] -> outT[jt*128+p, g*512+t] -> [T, D] -> [B, S, D]
    outT = acc.transpose(2, 0, 1, 3).reshape(D, T)
    out = np.ascontiguousarray(outT.T).reshape(B, S, D).astype(np.float32)
    return out

